# revision 7
# baseline (speedup 1.0000x reference)
"""Trainium2 Bass kernel: transformer block (causal MHA + dense top-2-gated MoE FFN).

Problem: nn_Block_24541443129820  (B=8, T=1024, D=768, H=12, DH=64, E=16, DFF=3072)

Sharding: data-parallel over batch. B == n_cores == 8, so each NeuronCore runs
the complete block (LN1 -> QKV -> causal attention -> proj+residual -> LN2 ->
router/top-2 gate -> all 16 experts, dense -> gated combine + double residual)
on one [1024, 768] batch slice. Weights are replicated to every core; the host
splits x on batch and stacks the per-core outputs. This is perfectly load
balanced and needs no collectives.

Per-core layout strategy:
 - activations live in "N-layout" [token-partition, feature-free] for LN/softmax
   (free-dim reductions) and in "T-layout" [feature-partition, token-free] when
   they feed matmuls as the stationary operand; PE transposes bridge the two.
 - all large matmuls run as float32r (1 PE cycle/row for free dim >= 256, i.e.
   bf16 speed with ~tf32 precision); attention probabilities/V use bf16.
 - expert MLPs: h1 = w1[e].T-slices x zT -> PSUM -> Gelu(+b1) on ACT -> gT in
   T-layout feeds h2 which accumulates K=DFF in PSUM; gate applied on eviction
   via one fused DVE scalar_tensor_tensor per chunk: acc += gate[t,e] * psum.
   The b2 term is folded in exactly via a tiny gateT @ b2 matmul that
   initializes the accumulator.
"""
import math
import sys

for _p in ("/opt/trn_rl_repo", "/root/.axon_site/_ro/trn_rl_repo"):
    if _p not in sys.path:
        sys.path.append(_p)

from contextlib import ExitStack
from dataclasses import dataclass

import numpy as np

import concourse.bass as bass
import concourse.tile as tile
from concourse import mybir

AF = mybir.ActivationFunctionType
OP = mybir.AluOpType
F32 = mybir.dt.float32
F32R = mybir.dt.float32r
BF16 = mybir.dt.bfloat16
P = 128


@dataclass(frozen=True)
class Cfg:
    T: int = 1024
    D: int = 768
    H: int = 12
    DH: int = 64
    E: int = 16
    DFF: int = 3072
    eps: float = 1e-5
    exact_gelu: bool = True  # False -> Tanh in place of Gelu (CoreSim lacks Gelu)


def _chunks(n, step=512):
    out = []
    off = 0
    while off < n:
        sz = min(step, n - off)
        out.append((off, sz))
        off += sz
    return out


def _bcast_ap(src_1d, parts):
    """DRAM [N] -> AP that a DMA reads as [parts, N] (partition-replicated)."""
    return bass.AP(
        tensor=src_1d.tensor,
        offset=src_1d.offset,
        ap=[[0, parts]] + [list(d) for d in src_1d.ap],
    )


def declare_io(nc: bass.Bass, c: Cfg):
    D3 = 3 * c.D
    io = {
        "x": nc.dram_tensor("x", [c.T, c.D], F32, kind="ExternalInput").ap(),
        "ln1_w": nc.dram_tensor("ln1_w", [c.D], F32, kind="ExternalInput").ap(),
        "ln1_b": nc.dram_tensor("ln1_b", [c.D], F32, kind="ExternalInput").ap(),
        "qkv_w": nc.dram_tensor("qkv_w", [c.D, D3], F32R, kind="ExternalInput").ap(),
        "qkv_b": nc.dram_tensor("qkv_b", [D3], F32, kind="ExternalInput").ap(),
        "proj_w": nc.dram_tensor("proj_w", [c.D, c.D], F32R, kind="ExternalInput").ap(),
        "proj_b": nc.dram_tensor("proj_b", [c.D], F32, kind="ExternalInput").ap(),
        "ln2_w": nc.dram_tensor("ln2_w", [c.D], F32, kind="ExternalInput").ap(),
        "ln2_b": nc.dram_tensor("ln2_b", [c.D], F32, kind="ExternalInput").ap(),
        "router_w": nc.dram_tensor("router_w", [c.D, c.E], F32, kind="ExternalInput").ap(),
        "w1": nc.dram_tensor("w1", [c.E, c.D, c.DFF], F32R, kind="ExternalInput").ap(),
        "b1": nc.dram_tensor("b1", [c.E, c.DFF], F32, kind="ExternalInput").ap(),
        "w2": nc.dram_tensor("w2", [c.E, c.DFF, c.D], F32R, kind="ExternalInput").ap(),
        "b2": nc.dram_tensor("b2", [c.E, c.D], F32R, kind="ExternalInput").ap(),
        "tri": nc.dram_tensor("tri", [P, P], BF16, kind="ExternalInput").ap(),
        "ident": nc.dram_tensor("ident", [P, P], F32, kind="ExternalInput").ap(),
        "out": nc.dram_tensor("out", [c.T, c.D], F32, kind="ExternalOutput").ap(),
    }
    return io


def _emit_ln(nc, stat, src, dst, w_b, b_b, eps_t, c):
    """dst = LN(src) * w + b, rowwise over the free dim (size D)."""
    SG = math.gcd(512, c.D)
    NSG = c.D // SG
    st = stat.tile([P, NSG, 6], F32, name="bnst", tag="bnst")
    for s in range(NSG):
        nc.vector.bn_stats(st[:, s, :], src[:, SG * s : SG * (s + 1)])
    mv = stat.tile([P, 2], F32, name="bnmv", tag="bnmv")
    nc.vector.bn_aggr(mv, st)
    rstd = stat.tile([P, 1], F32, name="rstd", tag="rstd")
    nc.scalar.activation(rstd, mv[:, 1:2], AF.Sqrt, bias=eps_t)
    nc.vector.reciprocal(rstd, rstd)
    nc.vector.tensor_scalar(
        out=dst, in0=src, scalar1=mv[:, 0:1], scalar2=rstd,
        op0=OP.subtract, op1=OP.mult,
    )
    nc.vector.tensor_mul(dst, dst, w_b)
    nc.vector.tensor_add(dst, dst, b_b)


def emit_block(tc: tile.TileContext, c: Cfg, io):
    nc = tc.nc
    TT = c.T // P           # token tiles
    KD = c.D // P           # model-dim k-tiles
    JD = c.DFF // P         # dff tiles
    JSPLIT = 2 if JD % 2 == 0 and JD > 1 else 1
    JH = JD // JSPLIT       # dff tiles per dff-half
    QW = min(512, c.T)      # MoE token-group width
    NHALF = c.T // QW
    TSUB = QW // P
    NQ = min(512, c.T)      # attention q-chunk width
    NCH = c.T // NQ
    HPT = P // c.DH         # heads per qT/kT partition tile
    QKT = (c.H * c.DH) // P  # qT (or kT) partition tiles
    DQK = 2 * c.H * c.DH
    dch = _chunks(c.D, 512)
    gelu_af = AF.Gelu if c.exact_gelu else AF.Tanh
    assert c.T % P == 0 and c.D % P == 0 and c.DFF % P == 0
    assert (c.H * c.DH) % P == 0 and c.DH <= P and P % c.DH == 0
    assert all(sz % c.DH == 0 for _, sz in dch)
    assert c.E >= 8  # vector.max needs >= 8 candidates

    with ExitStack() as ctx0:
        const = ctx0.enter_context(tc.tile_pool(name="const", bufs=1))
        ident_t = const.tile([P, P], F32, name="ident_t")
        nc.sync.dma_start(ident_t, io["ident"])
        tri_t = const.tile([P, P], BF16, name="tri_t")
        nc.sync.dma_start(tri_t, io["tri"])
        eps_t = const.tile([P, 1], F32, name="eps_t")
        nc.vector.memset(eps_t, c.eps)

        # b1 as per-partition columns: [P, e, j] = b1[e, 128j + p]
        b1_sb = const.tile([P, c.E, JD], F32, name="b1_sb")
        nc.sync.dma_start(b1_sb, io["b1"].rearrange("e (j p) -> p e j", p=P))

        persistX = ctx0.enter_context(tc.tile_pool(name="persistX", bufs=1))
        X = persistX.tile([P, TT, c.D], F32, name="X")
        for i in range(TT):
            nc.sync.dma_start(X[:, i, :], io["x"][P * i : P * (i + 1), :])

        # ================= attention =================
        with ExitStack() as actx:
            aouter = actx.enter_context(tc.tile_pool(name="attn_outer", bufs=1))
            QT = aouter.tile([P, QKT, c.T], F32R, name="QT")
            KTt = aouter.tile([P, QKT, c.T], F32R, name="KTt")
            VEXT = aouter.tile([P, TT, c.H, c.DH + 1], BF16, name="VEXT")
            Y = aouter.tile([P, TT, c.D], F32, name="Y")

            # ---- LN1 + transpose h -> hT + QKV matmuls ----
            with ExitStack() as qctx:
                hpool = qctx.enter_context(tc.tile_pool(name="hpool", bufs=3))
                htp = qctx.enter_context(tc.tile_pool(name="htp", bufs=1))
                HT = htp.tile([P, KD, c.T], F32R, name="HT")
                wpool = qctx.enter_context(tc.tile_pool(name="qkvwp", bufs=1))
                stat = qctx.enter_context(tc.tile_pool(name="stat1", bufs=4))
                cst1 = qctx.enter_context(tc.tile_pool(name="cst1", bufs=1))
                ln1w_b = cst1.tile([P, c.D], F32, name="ln1w_b")
                nc.gpsimd.dma_start(ln1w_b, _bcast_ap(io["ln1_w"], P))
                ln1b_b = cst1.tile([P, c.D], F32, name="ln1b_b")
                nc.gpsimd.dma_start(ln1b_b, _bcast_ap(io["ln1_b"], P))
                vbias_b = cst1.tile([P, c.D], F32, name="vbias_b")
                nc.gpsimd.dma_start(vbias_b, _bcast_ap(io["qkv_b"][DQK : DQK + c.D], P))
                # qkv_b for q,k as per-partition columns: col j = qkv_b[128j:128(j+1)]
                qkvbT = cst1.tile([P, DQK // P], F32, name="qkvbT")
                nc.sync.dma_start(qkvbT, io["qkv_b"][0:DQK].rearrange("(j p) -> p j", p=P))
                ptr = qctx.enter_context(tc.tile_pool(name="ptr1", bufs=4, space="PSUM"))
                pmm = qctx.enter_context(tc.tile_pool(name="pmm1", bufs=4, space="PSUM"))

                for i in range(TT):
                    h = hpool.tile([P, c.D], F32, name="h", tag="h")
                    _emit_ln(nc, stat, X[:, i, :], h, ln1w_b, ln1b_b, eps_t, c)
                    for k in range(KD):
                        pt = ptr.tile([P, P], F32, name="pt1", tag="pt1")
                        nc.tensor.transpose(pt, h[:, P * k : P * (k + 1)], ident_t)
                        nc.scalar.copy(HT[:, k, P * i : P * (i + 1)], pt)

                # qT / kT: out[dout_tile, tq] = qkv_w[:, tile].T @ hT
                for j in range(DQK // P):
                    wcol = []
                    for k in range(KD):
                        wt = wpool.tile([P, P], F32R, name="wqk", tag="wqk", bufs=KD + 2)
                        nc.sync.dma_start(
                            wt, io["qkv_w"][P * k : P * (k + 1), P * j : P * (j + 1)]
                        )
                        wcol.append(wt)
                    dst = QT if j < QKT else KTt
                    jj = j % QKT
                    for cc in range(NCH):
                        ps = pmm.tile([P, NQ], F32, name="ps_qk", tag="ps_qk")
                        for k in range(KD):
                            nc.tensor.matmul(
                                ps,
                                lhsT=wcol[k],
                                rhs=HT[:, k, NQ * cc : NQ * (cc + 1)],
                                start=(k == 0), stop=(k == KD - 1),
                            )
                        nc.scalar.activation(
                            dst[:, jj, NQ * cc : NQ * (cc + 1)], ps, AF.Identity,
                            bias=qkvbT[:, j : j + 1],
                        )

                # v (+bias) in N-layout, scattered into VEXT[:, :, h, 0:DH]
                for off, sz in dch:
                    wv = []
                    for k in range(KD):
                        wt = wpool.tile([P, 512], F32R, name="wv", tag="wv", bufs=KD + 2)
                        nc.sync.dma_start(
                            wt[:, :sz],
                            io["qkv_w"][P * k : P * (k + 1), DQK + off : DQK + off + sz],
                        )
                        wv.append(wt)
                    nh = sz // c.DH
                    h0 = off // c.DH
                    for i in range(TT):
                        ps = pmm.tile([P, sz], F32, name="ps_v", tag="ps_qk")
                        for k in range(KD):
                            nc.tensor.matmul(
                                ps,
                                lhsT=HT[:, k, P * i : P * (i + 1)],
                                rhs=wv[k][:, :sz],
                                start=(k == 0), stop=(k == KD - 1),
                            )
                        nc.vector.tensor_add(
                            VEXT[:, i, h0 : h0 + nh, 0 : c.DH],
                            ps.rearrange("p (h d) -> p h d", d=c.DH),
                            vbias_b[:, off : off + sz].rearrange(
                                "p (h d) -> p h d", d=c.DH
                            ),
                        )
                nc.vector.memset(VEXT[:, :, :, c.DH : c.DH + 1], 1.0)

            # ---- heads: scoresT -> exp -> causal mask -> A@V (+sums) ----
            with ExitStack() as hctx:
                apool = hctx.enter_context(tc.tile_pool(name="apool", bufs=2))
                small = hctx.enter_context(tc.tile_pool(name="asmall", bufs=6))
                pscore = hctx.enter_context(
                    tc.tile_pool(name="pscore", bufs=2, space="PSUM")
                )
                pav = hctx.enter_context(tc.tile_pool(name="pav", bufs=4, space="PSUM"))
                inv_sqrt_dh = 1.0 / math.sqrt(c.DH)
                for hh in range(c.H):
                    at = apool.tile([P, TT, c.T], BF16, name="at", tag="at")
                    pt_i = hh // HPT
                    po = (hh % HPT) * c.DH
                    for t in range(TT):
                        for cc in range(NCH):
                            if NQ * (cc + 1) <= P * t:
                                continue  # chunk fully in the causal-masked region
                            ps = pscore.tile([P, NQ], F32, name="ps_s", tag="ps_s")
                            nc.tensor.matmul(
                                ps,
                                lhsT=KTt[po : po + c.DH, pt_i, P * t : P * (t + 1)],
                                rhs=QT[po : po + c.DH, pt_i, NQ * cc : NQ * (cc + 1)],
                                start=True, stop=True,
                            )
                            nc.scalar.activation(
                                at[:, t, NQ * cc : NQ * (cc + 1)], ps, AF.Exp,
                                scale=inv_sqrt_dh,
                            )
                        # diagonal 128x128 block: zero out k > q
                        nc.vector.tensor_mul(
                            at[:, t, P * t : P * (t + 1)],
                            at[:, t, P * t : P * (t + 1)],
                            tri_t,
                        )
                    for i in range(TT):
                        pv = pav.tile([P, c.DH + 1], F32, name="pv", tag="pv")
                        for t in range(i + 1):
                            nc.tensor.matmul(
                                pv,
                                lhsT=at[:, t, P * i : P * (i + 1)],
                                rhs=VEXT[:, t, hh, :],
                                start=(t == 0), stop=(t == i),
                            )
                        rc = small.tile([P, 1], F32, name="rc", tag="rc")
                        nc.vector.reciprocal(rc, pv[:, c.DH : c.DH + 1])
                        nc.scalar.activation(
                            Y[:, i, c.DH * hh : c.DH * (hh + 1)], pv[:, 0 : c.DH],
                            AF.Copy, scale=rc,
                        )

            # ---- y -> ynT, proj, residual into X ----
            with ExitStack() as pctx:
                ynp = pctx.enter_context(tc.tile_pool(name="ynp", bufs=1))
                YNT = ynp.tile([P, KD, c.T], F32R, name="YNT")
                pwpool = pctx.enter_context(tc.tile_pool(name="pwpool", bufs=1))
                ptr2 = pctx.enter_context(tc.tile_pool(name="ptr2", bufs=4, space="PSUM"))
                cst2 = pctx.enter_context(tc.tile_pool(name="cst2", bufs=1))
                projb_b = cst2.tile([P, c.D], F32, name="projb_b")
                nc.gpsimd.dma_start(projb_b, _bcast_ap(io["proj_b"], P))
                pmm2 = pctx.enter_context(tc.tile_pool(name="pmm2", bufs=4, space="PSUM"))
                for i in range(TT):
                    for k in range(KD):
                        pt = ptr2.tile([P, P], F32, name="pt2", tag="pt2")
                        nc.tensor.transpose(pt, Y[:, i, P * k : P * (k + 1)], ident_t)
                        nc.scalar.copy(YNT[:, k, P * i : P * (i + 1)], pt)
                    nc.vector.tensor_add(X[:, i, :], X[:, i, :], projb_b)
                for off, sz in dch:
                    pw = []
                    for k in range(KD):
                        wt = pwpool.tile([P, 512], F32R, name="pw", tag="pw", bufs=KD + 2)
                        nc.sync.dma_start(
                            wt[:, :sz],
                            io["proj_w"][P * k : P * (k + 1), off : off + sz],
                        )
                        pw.append(wt)
                    for i in range(TT):
                        ps = pmm2.tile([P, sz], F32, name="ps_p", tag="ps_p")
                        for k in range(KD):
                            nc.tensor.matmul(
                                ps,
                                lhsT=YNT[:, k, P * i : P * (i + 1)],
                                rhs=pw[k][:, :sz],
                                start=(k == 0), stop=(k == KD - 1),
                            )
                        nc.vector.scalar_tensor_tensor(
                            out=X[:, i, off : off + sz], in0=ps, scalar=1.0,
                            in1=X[:, i, off : off + sz],
                            op0=OP.mult, op1=OP.add,
                        )

        # ================= LN2 -> zT; r = x1 + z into X =================
        ztp = ctx0.enter_context(tc.tile_pool(name="ztp", bufs=1))
        ZT = ztp.tile([P, KD, c.T], F32R, name="ZT")
        with ExitStack() as lctx:
            zpool = lctx.enter_context(tc.tile_pool(name="zpool", bufs=3))
            stat2 = lctx.enter_context(tc.tile_pool(name="stat2", bufs=4))
            ptr3 = lctx.enter_context(tc.tile_pool(name="ptr3", bufs=4, space="PSUM"))
            cst3 = lctx.enter_context(tc.tile_pool(name="cst3", bufs=1))
            ln2w_b = cst3.tile([P, c.D], F32, name="ln2w_b")
            nc.gpsimd.dma_start(ln2w_b, _bcast_ap(io["ln2_w"], P))
            ln2b_b = cst3.tile([P, c.D], F32, name="ln2b_b")
            nc.gpsimd.dma_start(ln2b_b, _bcast_ap(io["ln2_b"], P))
            for i in range(TT):
                z = zpool.tile([P, c.D], F32, name="z", tag="z")
                _emit_ln(nc, stat2, X[:, i, :], z, ln2w_b, ln2b_b, eps_t, c)
                for k in range(KD):
                    pt = ptr3.tile([P, P], F32, name="pt3", tag="pt3")
                    nc.tensor.transpose(pt, z[:, P * k : P * (k + 1)], ident_t)
                    nc.scalar.copy(ZT[:, k, P * i : P * (i + 1)], pt)
                nc.vector.tensor_add(X[:, i, :], X[:, i, :], z)

        # ================= router, top-2 gate, b2-init of ACC =================
        gatep = ctx0.enter_context(tc.tile_pool(name="gatep", bufs=1))
        GATE = gatep.tile([P, TT, c.E], F32, name="GATE")
        GATET = gatep.tile([c.E, c.T], F32R, name="GATET")
        accp = ctx0.enter_context(tc.tile_pool(name="accp", bufs=1))
        ACC = accp.tile([P, TT, c.D], F32, name="ACC")
        with ExitStack() as rctx:
            rwp = rctx.enter_context(tc.tile_pool(name="rwp", bufs=1))
            RW = rwp.tile([P, KD, c.E], F32, name="RW")
            nc.sync.dma_start(RW, io["router_w"].rearrange("(k p) e -> p k e", p=P))
            B2 = rwp.tile([c.E, c.D], F32R, name="B2")
            nc.sync.dma_start(B2, io["b2"])
            rsmall = rctx.enter_context(tc.tile_pool(name="rsmall", bufs=4))
            prr = rctx.enter_context(tc.tile_pool(name="prr", bufs=2, space="PSUM"))
            ptg = rctx.enter_context(tc.tile_pool(name="ptg", bufs=2, space="PSUM"))
            pb2 = rctx.enter_context(tc.tile_pool(name="pb2", bufs=4, space="PSUM"))
            for i in range(TT):
                ps = prr.tile([P, c.E], F32, name="ps_r", tag="ps_r")
                for k in range(KD):
                    nc.tensor.matmul(
                        ps,
                        lhsT=ZT[:, k, P * i : P * (i + 1)].bitcast(F32),
                        rhs=RW[:, k, :],
                        start=(k == 0), stop=(k == KD - 1),
                    )
                mx = rsmall.tile([P, 1], F32, name="mx", tag="mx")
                nc.vector.reduce_max(mx, ps, axis=mybir.AxisListType.X)
                negmx = rsmall.tile([P, 1], F32, name="negmx", tag="negmx")
                nc.vector.tensor_scalar_mul(negmx, mx, -1.0)
                probs = rsmall.tile([P, c.E], F32, name="probs", tag="probs")
                sums = rsmall.tile([P, 1], F32, name="sums", tag="sums")
                nc.scalar.activation(
                    probs, ps, AF.Exp, bias=negmx, accum_out=sums
                )
                rcp = rsmall.tile([P, 1], F32, name="rcp", tag="rcp")
                nc.vector.reciprocal(rcp, sums)
                nc.vector.tensor_scalar_mul(probs, probs, rcp)
                m8 = rsmall.tile([P, 8], F32, name="m8", tag="m8")
                nc.vector.max(m8, probs)
                nc.vector.tensor_scalar(
                    out=GATE[:, i, :], in0=probs, scalar1=m8[:, 1:2], scalar2=None,
                    op0=OP.is_ge,
                )
                nc.vector.tensor_mul(GATE[:, i, :], GATE[:, i, :], probs)
                pt = ptg.tile([c.E, P], F32, name="ptg", tag="ptg")
                nc.tensor.transpose(pt, GATE[:, i, :], ident_t)
                nc.scalar.copy(GATET[:, P * i : P * (i + 1)], pt)
            # ACC = gate @ b2  (exact b2 contribution: sum_e gate[t,e] * b2[e,:])
            for i in range(TT):
                for off, sz in dch:
                    ps = pb2.tile([P, 512], F32, name="ps_b2", tag="ps_b2")
                    nc.tensor.matmul(
                        ps[:, :sz],
                        lhsT=GATET[:, P * i : P * (i + 1)],
                        rhs=B2[:, off : off + sz],
                        start=True, stop=True,
                    )
                    nc.any.tensor_copy(ACC[:, i, off : off + sz], ps[:, :sz])

        # ================= MoE experts =================
        with ExitStack() as mctx:
            w1p = mctx.enter_context(tc.tile_pool(name="w1p", bufs=1))
            w2p = mctx.enter_context(tc.tile_pool(name="w2p", bufs=1))
            gtp = mctx.enter_context(tc.tile_pool(name="gtp", bufs=4))
            ph1p = mctx.enter_context(tc.tile_pool(name="ph1p", bufs=2, space="PSUM"))
            pacc = mctx.enter_context(tc.tile_pool(name="pacc", bufs=4, space="PSUM"))
            DFFSTR = c.D * c.DFF  # elements per expert in w1

            for e in range(c.E):
                for dhalf in range(JSPLIT):
                    w1t = []
                    w2t = []
                    for j in range(JH):
                        jj = dhalf * JH + j
                        w1_ = w1p.tile([P, KD, P], F32R, name="w1t", tag="w1t", bufs=JH + 1)
                        # [p, k, cij] <- w1[e, 128k + p, 128jj + cij]
                        src = bass.AP(
                            tensor=io["w1"].tensor,
                            offset=e * DFFSTR + P * jj,
                            ap=[[c.DFF, P], [P * c.DFF, KD], [1, P]],
                        )
                        nc.sync.dma_start(w1_, src)
                        w1t.append(w1_)
                        w2_ = w2p.tile([P, c.D], F32R, name="w2t", tag="w2t", bufs=JH + 1)
                        nc.sync.dma_start(w2_, io["w2"][e, P * jj : P * (jj + 1), :])
                        w2t.append(w2_)
                    for half in range(NHALF):
                        # h1: all JH gelu tiles of this token-group materialized
                        gts = []
                        for j in range(JH):
                            jj = dhalf * JH + j
                            ph = ph1p.tile([P, QW], F32, name="ph1", tag="ph1")
                            for k in range(KD):
                                nc.tensor.matmul(
                                    ph,
                                    lhsT=w1t[j][:, k, :],
                                    rhs=ZT[:, k, QW * half : QW * (half + 1)],
                                    start=(k == 0), stop=(k == KD - 1),
                                )
                            g = gtp.tile([P, QW], F32R, name="g", tag="g", bufs=JH + 1)
                            nc.scalar.activation(
                                g, ph, gelu_af, bias=b1_sb[:, e, jj : jj + 1]
                            )
                            gts.append(g)
                        # h2: one PSUM accumulation group per (token-tile, chunk)
                        for i in range(TSUB):
                            ti = half * TSUB + i
                            for ci, (off, sz) in enumerate(dch):
                                ps = pacc.tile([P, 512], F32, name="pacc", tag="pacc")
                                for j in range(JH):
                                    nc.tensor.matmul(
                                        ps[:, :sz],
                                        lhsT=gts[j][:, P * i : P * (i + 1)],
                                        rhs=w2t[j][:, off : off + sz],
                                        start=(j == 0), stop=(j == JH - 1),
                                    )
                                nc.vector.scalar_tensor_tensor(
                                    out=ACC[:, ti, off : off + sz],
                                    in0=ps[:, :sz],
                                    scalar=GATE[:, ti, e : e + 1],
                                    in1=ACC[:, ti, off : off + sz],
                                    op0=OP.mult, op1=OP.add,
                                )

        # ================= out = r + yff =================
        with ExitStack() as octx:
            op = octx.enter_context(tc.tile_pool(name="outp", bufs=3))
            for i in range(TT):
                ot = op.tile([P, c.D], F32, name="ot", tag="ot")
                nc.vector.tensor_add(ot, X[:, i, :], ACC[:, i, :])
                nc.sync.dma_start(io["out"][P * i : P * (i + 1), :], ot)


def build(c: Cfg | None = None) -> bass.Bass:
    from concourse import bacc

    c = c or Cfg()
    nc = bacc.Bacc("TRN2", target_bir_lowering=False, debug=False)
    io = declare_io(nc, c)
    with tile.TileContext(nc) as tc:
        emit_block(tc, c, io)
    nc.compile()
    return nc


def make_consts(c: Cfg | None = None):
    import ml_dtypes

    c = c or Cfg()
    tri = np.triu(np.ones((P, P), np.float32)).astype(ml_dtypes.bfloat16)
    ident = np.eye(P, dtype=np.float32)
    return {"tri": tri, "ident": ident}


_BUILT: bass.Bass | None = None
_RUNNER = None

N_CORES = 8
_IN_NAMES = [
    "x", "ln1_w", "ln1_b", "qkv_w", "qkv_b", "proj_w", "proj_b",
    "ln2_w", "ln2_b", "router_w", "w1", "b1", "w2", "b2",
]


def get_runner(c: Cfg | None = None, n_cores: int = N_CORES):
    """Build (once) and return (fn, in_names, out_names, out_shapes).

    fn takes per-core-concatenated arrays (inputs then zero output buffers),
    runs the NEFF on n_cores devices via shard_map, returns output arrays.
    """
    global _BUILT, _RUNNER
    if _RUNNER is not None:
        return _RUNNER
    import jax
    from jax.experimental.shard_map import shard_map
    from jax.sharding import Mesh, PartitionSpec
    from concourse import bass2jax
    from concourse import mybir as _mb

    c = c or Cfg()
    if _BUILT is None:
        _BUILT = build(c)
    nc = _BUILT
    bass2jax.install_neuronx_cc_hook()
    assert nc.dbg_addr is None
    partition_name = nc.partition_id_tensor.name if nc.partition_id_tensor else None
    in_names, out_names, out_avals = [], [], []
    for alloc in nc.m.functions[0].allocations:
        if not isinstance(alloc, _mb.MemoryLocationSet):
            continue
        name = alloc.memorylocations[0].name
        if alloc.kind == "ExternalInput":
            if name != partition_name:
                in_names.append(name)
        elif alloc.kind == "ExternalOutput":
            out_avals.append(
                jax.core.ShapedArray(tuple(alloc.tensor_shape), _mb.dt.np(alloc.dtype))
            )
            out_names.append(name)
    n_params = len(in_names)
    all_in = tuple(in_names) + tuple(out_names)
    if partition_name is not None:
        all_in = all_in + (partition_name,)

    def _body(*args):
        ops = list(args)
        if partition_name is not None:
            ops.append(bass2jax.partition_id_tensor())
        outs = bass2jax._bass_exec_p.bind(
            *ops,
            out_avals=tuple(out_avals),
            in_names=all_in,
            out_names=tuple(out_names),
            lowering_input_output_aliases=(),
            sim_require_finite=True,
            sim_require_nnan=True,
            nc=nc,
        )
        return tuple(outs)

    devices = jax.devices()[:n_cores]
    mesh = Mesh(np.asarray(devices), ("core",))
    nio = n_params + len(out_names)
    fn = jax.jit(
        shard_map(
            _body,
            mesh=mesh,
            in_specs=(PartitionSpec("core"),) * nio,
            out_specs=(PartitionSpec("core"),) * len(out_names),
            check_rep=False,
        ),
        keep_unused=True,
    )
    out_shapes = [tuple(a.shape) for a in out_avals]
    out_dtypes = [a.dtype for a in out_avals]
    _RUNNER = (fn, in_names, out_names, out_shapes, out_dtypes)
    return _RUNNER


def _concat_inputs(arrs, consts, c: Cfg, in_names, out_shapes, out_dtypes, n_cores=N_CORES):
    """Per-core replicated/sharded inputs, concatenated on axis 0 for shard_map."""
    x = arrs["x"]
    per_name = {}
    for nm in in_names:
        if nm == "x":
            per_name[nm] = np.ascontiguousarray(x.reshape(n_cores * c.T, c.D))
        else:
            src = consts[nm] if nm in consts else arrs[nm]
            per_name[nm] = np.concatenate([src] * n_cores, axis=0)
    ins = [per_name[nm] for nm in in_names]
    zouts = [
        np.zeros((n_cores * s[0], *s[1:]), dt) for s, dt in zip(out_shapes, out_dtypes)
    ]
    return ins, zouts


def kernel(**inputs) -> np.ndarray:
    c = Cfg()
    arrs = {
        k: np.ascontiguousarray(np.asarray(v, dtype=np.float32))
        for k, v in inputs.items()
    }
    x = arrs["x"]  # [B, T, D]
    B = x.shape[0]
    assert B == N_CORES and x.shape[1] == c.T and x.shape[2] == c.D

    fn, in_names, out_names, out_shapes, out_dtypes = get_runner(c)
    consts = make_consts(c)
    ins, zouts = _concat_inputs(arrs, consts, c, in_names, out_shapes, out_dtypes)
    outs = fn(*ins, *zouts)
    out = np.asarray(outs[out_names.index("out")]).reshape(N_CORES, c.T, c.D)
    return out.astype(np.float32)


def _warmup():
    """Compile the NEFF + load executables at import so kernel() calls are fast."""
    try:
        c = Cfg()
        fn, in_names, out_names, out_shapes, out_dtypes = get_runner(c)
        rng = np.random.default_rng(0)
        dummy = {
            "x": np.zeros((N_CORES, c.T, c.D), np.float32),
            "ln1_w": np.ones(c.D, np.float32), "ln1_b": np.zeros(c.D, np.float32),
            "qkv_w": np.zeros((c.D, 3 * c.D), np.float32),
            "qkv_b": np.zeros(3 * c.D, np.float32),
            "proj_w": np.zeros((c.D, c.D), np.float32),
            "proj_b": np.zeros(c.D, np.float32),
            "ln2_w": np.ones(c.D, np.float32), "ln2_b": np.zeros(c.D, np.float32),
            "router_w": np.zeros((c.D, c.E), np.float32),
            "w1": np.zeros((c.E, c.D, c.DFF), np.float32),
            "b1": np.zeros((c.E, c.DFF), np.float32),
            "w2": np.zeros((c.E, c.DFF, c.D), np.float32),
            "b2": np.zeros((c.E, c.D), np.float32),
        }
        consts = make_consts(c)
        ins, zouts = _concat_inputs(dummy, consts, c, in_names, out_shapes, out_dtypes)
        import jax
        jax.block_until_ready(fn(*ins, *zouts))
    except Exception:
        import traceback
        traceback.print_exc()


import os as _os

if not _os.environ.get("KERNEL_NO_WARMUP"):
    _warmup()


# revision 9
# speedup vs baseline: 23.7898x; 23.7898x over previous
"""Trainium2 Bass kernel: transformer block (causal MHA + dense top-2-gated MoE FFN).

Problem: nn_Block_24541443129820  (B=8, T=1024, D=768, H=12, DH=64, E=16, DFF=3072)

Sharding: data-parallel over batch. B == n_cores == 8, so each NeuronCore runs
the complete block (LN1 -> QKV -> causal attention -> proj+residual -> LN2 ->
router/top-2 gate -> all 16 experts, dense -> gated combine + double residual)
on one [1024, 768] batch slice. Weights are replicated to every core; the host
splits x on batch and stacks the per-core outputs. This is perfectly load
balanced and needs no collectives.

Per-core layout strategy:
 - activations live in "N-layout" [token-partition, feature-free] for LN/softmax
   (free-dim reductions) and in "T-layout" [feature-partition, token-free] when
   they feed matmuls as the stationary operand; PE transposes bridge the two.
 - all large matmuls run as float32r (1 PE cycle/row for free dim >= 256, i.e.
   bf16 speed with ~tf32 precision); attention probabilities/V use bf16.
 - expert MLPs: h1 = w1[e].T-slices x zT -> PSUM -> Gelu(+b1) on ACT -> gT in
   T-layout feeds h2 which accumulates K=DFF in PSUM; gate applied on eviction
   via one fused DVE scalar_tensor_tensor per chunk: acc += gate[t,e] * psum.
   The b2 term is folded in exactly via a tiny gateT @ b2 matmul that
   initializes the accumulator.
"""
import math
import sys

for _p in ("/opt/trn_rl_repo", "/root/.axon_site/_ro/trn_rl_repo"):
    if _p not in sys.path:
        sys.path.append(_p)

from contextlib import ExitStack
from dataclasses import dataclass

import numpy as np

import concourse.bass as bass
import concourse.tile as tile
from concourse import mybir

AF = mybir.ActivationFunctionType
OP = mybir.AluOpType
F32 = mybir.dt.float32
F32R = mybir.dt.float32r
BF16 = mybir.dt.bfloat16
P = 128


@dataclass(frozen=True)
class Cfg:
    T: int = 1024
    D: int = 768
    H: int = 12
    DH: int = 64
    E: int = 16
    DFF: int = 3072
    eps: float = 1e-5
    exact_gelu: bool = True  # False -> Tanh in place of Gelu (CoreSim lacks Gelu)


def _chunks(n, step=512):
    out = []
    off = 0
    while off < n:
        sz = min(step, n - off)
        out.append((off, sz))
        off += sz
    return out


def _bcast_ap(src_1d, parts):
    """DRAM [N] -> AP that a DMA reads as [parts, N] (partition-replicated)."""
    return bass.AP(
        tensor=src_1d.tensor,
        offset=src_1d.offset,
        ap=[[0, parts]] + [list(d) for d in src_1d.ap],
    )


def declare_io(nc: bass.Bass, c: Cfg):
    D3 = 3 * c.D
    io = {
        "x": nc.dram_tensor("x", [c.T, c.D], F32, kind="ExternalInput").ap(),
        "ln1_w": nc.dram_tensor("ln1_w", [c.D], F32, kind="ExternalInput").ap(),
        "ln1_b": nc.dram_tensor("ln1_b", [c.D], F32, kind="ExternalInput").ap(),
        "qkv_w": nc.dram_tensor("qkv_w", [c.D, D3], F32R, kind="ExternalInput").ap(),
        "qkv_b": nc.dram_tensor("qkv_b", [D3], F32, kind="ExternalInput").ap(),
        "proj_w": nc.dram_tensor("proj_w", [c.D, c.D], F32R, kind="ExternalInput").ap(),
        "proj_b": nc.dram_tensor("proj_b", [c.D], F32, kind="ExternalInput").ap(),
        "ln2_w": nc.dram_tensor("ln2_w", [c.D], F32, kind="ExternalInput").ap(),
        "ln2_b": nc.dram_tensor("ln2_b", [c.D], F32, kind="ExternalInput").ap(),
        "router_w": nc.dram_tensor("router_w", [c.D, c.E], F32, kind="ExternalInput").ap(),
        "w1": nc.dram_tensor("w1", [c.E, c.D, c.DFF], F32R, kind="ExternalInput").ap(),
        "b1": nc.dram_tensor("b1", [c.E, c.DFF], F32, kind="ExternalInput").ap(),
        "w2": nc.dram_tensor("w2", [c.E, c.DFF, c.D], F32R, kind="ExternalInput").ap(),
        "b2": nc.dram_tensor("b2", [c.E, c.D], F32R, kind="ExternalInput").ap(),
        "tri": nc.dram_tensor("tri", [P, P], F32, kind="ExternalInput").ap(),
        "ident": nc.dram_tensor("ident", [P, P], F32, kind="ExternalInput").ap(),
        "out": nc.dram_tensor("out", [c.T, c.D], F32, kind="ExternalOutput").ap(),
    }
    return io


def _emit_ln(nc, stat, src, dst, w_b, b_b, eps_t, c):
    """dst = LN(src) * w + b, rowwise over the free dim (size D)."""
    SG = math.gcd(512, c.D)
    NSG = c.D // SG
    st = stat.tile([P, NSG, 6], F32, name="bnst", tag="bnst")
    for s in range(NSG):
        nc.vector.bn_stats(st[:, s, :], src[:, SG * s : SG * (s + 1)])
    mv = stat.tile([P, 2], F32, name="bnmv", tag="bnmv")
    nc.vector.bn_aggr(mv, st)
    rstd = stat.tile([P, 1], F32, name="rstd", tag="rstd")
    nc.scalar.activation(rstd, mv[:, 1:2], AF.Sqrt, bias=eps_t)
    nc.vector.reciprocal(rstd, rstd)
    nc.vector.tensor_scalar(
        out=dst, in0=src, scalar1=mv[:, 0:1], scalar2=rstd,
        op0=OP.subtract, op1=OP.mult,
    )
    nc.vector.tensor_mul(dst, dst, w_b)
    nc.vector.tensor_add(dst, dst, b_b)


def emit_block(tc: tile.TileContext, c: Cfg, io):
    nc = tc.nc
    TT = c.T // P           # token tiles
    KD = c.D // P           # model-dim k-tiles
    JD = c.DFF // P         # dff tiles
    JSPLIT = 2 if JD % 2 == 0 and JD > 1 else 1
    JH = JD // JSPLIT       # dff tiles per dff-half
    QW = min(512, c.T)      # MoE token-group width
    NHALF = c.T // QW
    TSUB = QW // P
    NQ = min(512, c.T)      # attention q-chunk width
    NCH = c.T // NQ
    HPT = P // c.DH         # heads per qT/kT partition tile
    QKT = (c.H * c.DH) // P  # qT (or kT) partition tiles
    DQK = 2 * c.H * c.DH
    dch = _chunks(c.D, 512)
    gelu_af = AF.Gelu if c.exact_gelu else AF.Tanh
    assert c.T % P == 0 and c.D % P == 0 and c.DFF % P == 0
    assert (c.H * c.DH) % P == 0 and c.DH <= P and P % c.DH == 0
    assert all(sz % c.DH == 0 for _, sz in dch)
    assert c.E >= 8  # vector.max needs >= 8 candidates

    with ExitStack() as ctx0:
        const = ctx0.enter_context(tc.tile_pool(name="const", bufs=1))
        ident_t = const.tile([P, P], F32, name="ident_t")
        nc.sync.dma_start(ident_t, io["ident"])
        tri_t = const.tile([P, P], F32, name="tri_t")
        nc.sync.dma_start(tri_t, io["tri"])
        eps_t = const.tile([P, 1], F32, name="eps_t")
        nc.vector.memset(eps_t, c.eps)

        # b1 as per-partition columns: [P, e, j] = b1[e, 128j + p]
        b1_sb = const.tile([P, c.E, JD], F32, name="b1_sb")
        nc.sync.dma_start(b1_sb, io["b1"].rearrange("e (j p) -> p e j", p=P))

        persistX = ctx0.enter_context(tc.tile_pool(name="persistX", bufs=1))
        X = persistX.tile([P, TT, c.D], F32, name="X")
        for i in range(TT):
            nc.sync.dma_start(X[:, i, :], io["x"][P * i : P * (i + 1), :])

        # ================= attention =================
        with ExitStack() as actx:
            aouter = actx.enter_context(tc.tile_pool(name="attn_outer", bufs=1))
            QT = aouter.tile([P, QKT, c.T], F32R, name="QT")
            KTt = aouter.tile([P, QKT, c.T], F32R, name="KTt")
            VEXT = aouter.tile([P, TT, c.H, c.DH + 1], F32, name="VEXT")
            Y = aouter.tile([P, TT, c.D], F32, name="Y")

            # ---- LN1 + transpose h -> hT + QKV matmuls ----
            with ExitStack() as qctx:
                hpool = qctx.enter_context(tc.tile_pool(name="hpool", bufs=3))
                htp = qctx.enter_context(tc.tile_pool(name="htp", bufs=1))
                HT = htp.tile([P, KD, c.T], F32R, name="HT")
                wpool = qctx.enter_context(tc.tile_pool(name="qkvwp", bufs=1))
                stat = qctx.enter_context(tc.tile_pool(name="stat1", bufs=4))
                cst1 = qctx.enter_context(tc.tile_pool(name="cst1", bufs=1))
                ln1w_b = cst1.tile([P, c.D], F32, name="ln1w_b")
                nc.gpsimd.dma_start(ln1w_b, _bcast_ap(io["ln1_w"], P))
                ln1b_b = cst1.tile([P, c.D], F32, name="ln1b_b")
                nc.gpsimd.dma_start(ln1b_b, _bcast_ap(io["ln1_b"], P))
                vbias_b = cst1.tile([P, c.D], F32, name="vbias_b")
                nc.gpsimd.dma_start(vbias_b, _bcast_ap(io["qkv_b"][DQK : DQK + c.D], P))
                # qkv_b for q,k as per-partition columns: col j = qkv_b[128j:128(j+1)]
                qkvbT = cst1.tile([P, DQK // P], F32, name="qkvbT")
                nc.sync.dma_start(qkvbT, io["qkv_b"][0:DQK].rearrange("(j p) -> p j", p=P))
                ptr = qctx.enter_context(tc.tile_pool(name="ptr1", bufs=4, space="PSUM"))
                pmm = qctx.enter_context(tc.tile_pool(name="pmm1", bufs=4, space="PSUM"))

                for i in range(TT):
                    h = hpool.tile([P, c.D], F32, name="h", tag="h")
                    _emit_ln(nc, stat, X[:, i, :], h, ln1w_b, ln1b_b, eps_t, c)
                    for k in range(KD):
                        pt = ptr.tile([P, P], F32, name="pt1", tag="pt1")
                        nc.tensor.transpose(pt, h[:, P * k : P * (k + 1)], ident_t)
                        nc.scalar.copy(HT[:, k, P * i : P * (i + 1)], pt)

                # qT / kT: out[dout_tile, tq] = qkv_w[:, tile].T @ hT
                for j in range(DQK // P):
                    wcol = []
                    for k in range(KD):
                        wt = wpool.tile([P, P], F32R, name="wqk", tag="wqk", bufs=KD + 2)
                        nc.sync.dma_start(
                            wt, io["qkv_w"][P * k : P * (k + 1), P * j : P * (j + 1)]
                        )
                        wcol.append(wt)
                    dst = QT if j < QKT else KTt
                    jj = j % QKT
                    for cc in range(NCH):
                        ps = pmm.tile([P, NQ], F32, name="ps_qk", tag="ps_qk")
                        for k in range(KD):
                            nc.tensor.matmul(
                                ps,
                                lhsT=wcol[k],
                                rhs=HT[:, k, NQ * cc : NQ * (cc + 1)],
                                start=(k == 0), stop=(k == KD - 1),
                            )
                        nc.scalar.activation(
                            dst[:, jj, NQ * cc : NQ * (cc + 1)], ps, AF.Identity,
                            bias=qkvbT[:, j : j + 1],
                        )

                # v (+bias) in N-layout, scattered into VEXT[:, :, h, 0:DH]
                for off, sz in dch:
                    wv = []
                    for k in range(KD):
                        wt = wpool.tile([P, 512], F32R, name="wv", tag="wv", bufs=KD + 2)
                        nc.sync.dma_start(
                            wt[:, :sz],
                            io["qkv_w"][P * k : P * (k + 1), DQK + off : DQK + off + sz],
                        )
                        wv.append(wt)
                    nh = sz // c.DH
                    h0 = off // c.DH
                    for i in range(TT):
                        ps = pmm.tile([P, sz], F32, name="ps_v", tag="ps_qk")
                        for k in range(KD):
                            nc.tensor.matmul(
                                ps,
                                lhsT=HT[:, k, P * i : P * (i + 1)],
                                rhs=wv[k][:, :sz],
                                start=(k == 0), stop=(k == KD - 1),
                            )
                        nc.vector.tensor_add(
                            VEXT[:, i, h0 : h0 + nh, 0 : c.DH],
                            ps.rearrange("p (h d) -> p h d", d=c.DH),
                            vbias_b[:, off : off + sz].rearrange(
                                "p (h d) -> p h d", d=c.DH
                            ),
                        )
                nc.vector.memset(VEXT[:, :, :, c.DH : c.DH + 1], 1.0)

            # ---- heads: scoresT -> exp -> causal mask -> A@V (+sums) ----
            with ExitStack() as hctx:
                apool = hctx.enter_context(tc.tile_pool(name="apool", bufs=1))
                small = hctx.enter_context(tc.tile_pool(name="asmall", bufs=6))
                pscore = hctx.enter_context(
                    tc.tile_pool(name="pscore", bufs=2, space="PSUM")
                )
                pav = hctx.enter_context(tc.tile_pool(name="pav", bufs=4, space="PSUM"))
                inv_sqrt_dh = 1.0 / math.sqrt(c.DH)
                for hh in range(c.H):
                    at = apool.tile([P, TT, c.T], F32, name="at", tag="at")
                    pt_i = hh // HPT
                    po = (hh % HPT) * c.DH
                    for t in range(TT):
                        for cc in range(NCH):
                            if NQ * (cc + 1) <= P * t:
                                continue  # chunk fully in the causal-masked region
                            ps = pscore.tile([P, NQ], F32, name="ps_s", tag="ps_s")
                            nc.tensor.matmul(
                                ps,
                                lhsT=KTt[po : po + c.DH, pt_i, P * t : P * (t + 1)],
                                rhs=QT[po : po + c.DH, pt_i, NQ * cc : NQ * (cc + 1)],
                                start=True, stop=True,
                            )
                            nc.scalar.activation(
                                at[:, t, NQ * cc : NQ * (cc + 1)], ps, AF.Exp,
                                scale=inv_sqrt_dh,
                            )
                        # diagonal 128x128 block: zero out k > q
                        nc.vector.tensor_mul(
                            at[:, t, P * t : P * (t + 1)],
                            at[:, t, P * t : P * (t + 1)],
                            tri_t,
                        )
                    for i in range(TT):
                        pv = pav.tile([P, c.DH + 1], F32, name="pv", tag="pv")
                        for t in range(i + 1):
                            nc.tensor.matmul(
                                pv,
                                lhsT=at[:, t, P * i : P * (i + 1)],
                                rhs=VEXT[:, t, hh, :],
                                start=(t == 0), stop=(t == i),
                            )
                        rc = small.tile([P, 1], F32, name="rc", tag="rc")
                        nc.vector.reciprocal(rc, pv[:, c.DH : c.DH + 1])
                        nc.scalar.activation(
                            Y[:, i, c.DH * hh : c.DH * (hh + 1)], pv[:, 0 : c.DH],
                            AF.Copy, scale=rc,
                        )

            # ---- y -> ynT, proj, residual into X ----
            with ExitStack() as pctx:
                ynp = pctx.enter_context(tc.tile_pool(name="ynp", bufs=1))
                YNT = ynp.tile([P, KD, c.T], F32R, name="YNT")
                pwpool = pctx.enter_context(tc.tile_pool(name="pwpool", bufs=1))
                ptr2 = pctx.enter_context(tc.tile_pool(name="ptr2", bufs=4, space="PSUM"))
                cst2 = pctx.enter_context(tc.tile_pool(name="cst2", bufs=1))
                projb_b = cst2.tile([P, c.D], F32, name="projb_b")
                nc.gpsimd.dma_start(projb_b, _bcast_ap(io["proj_b"], P))
                pmm2 = pctx.enter_context(tc.tile_pool(name="pmm2", bufs=4, space="PSUM"))
                for i in range(TT):
                    for k in range(KD):
                        pt = ptr2.tile([P, P], F32, name="pt2", tag="pt2")
                        nc.tensor.transpose(pt, Y[:, i, P * k : P * (k + 1)], ident_t)
                        nc.scalar.copy(YNT[:, k, P * i : P * (i + 1)], pt)
                    nc.vector.tensor_add(X[:, i, :], X[:, i, :], projb_b)
                for off, sz in dch:
                    pw = []
                    for k in range(KD):
                        wt = pwpool.tile([P, 512], F32R, name="pw", tag="pw", bufs=KD + 2)
                        nc.sync.dma_start(
                            wt[:, :sz],
                            io["proj_w"][P * k : P * (k + 1), off : off + sz],
                        )
                        pw.append(wt)
                    for i in range(TT):
                        ps = pmm2.tile([P, sz], F32, name="ps_p", tag="ps_p")
                        for k in range(KD):
                            nc.tensor.matmul(
                                ps,
                                lhsT=YNT[:, k, P * i : P * (i + 1)],
                                rhs=pw[k][:, :sz],
                                start=(k == 0), stop=(k == KD - 1),
                            )
                        nc.vector.scalar_tensor_tensor(
                            out=X[:, i, off : off + sz], in0=ps, scalar=1.0,
                            in1=X[:, i, off : off + sz],
                            op0=OP.mult, op1=OP.add,
                        )

        # ================= LN2 -> zT; r = x1 + z into X =================
        ztp = ctx0.enter_context(tc.tile_pool(name="ztp", bufs=1))
        ZT = ztp.tile([P, KD, c.T], F32R, name="ZT")
        with ExitStack() as lctx:
            zpool = lctx.enter_context(tc.tile_pool(name="zpool", bufs=3))
            stat2 = lctx.enter_context(tc.tile_pool(name="stat2", bufs=4))
            ptr3 = lctx.enter_context(tc.tile_pool(name="ptr3", bufs=4, space="PSUM"))
            cst3 = lctx.enter_context(tc.tile_pool(name="cst3", bufs=1))
            ln2w_b = cst3.tile([P, c.D], F32, name="ln2w_b")
            nc.gpsimd.dma_start(ln2w_b, _bcast_ap(io["ln2_w"], P))
            ln2b_b = cst3.tile([P, c.D], F32, name="ln2b_b")
            nc.gpsimd.dma_start(ln2b_b, _bcast_ap(io["ln2_b"], P))
            for i in range(TT):
                z = zpool.tile([P, c.D], F32, name="z", tag="z")
                _emit_ln(nc, stat2, X[:, i, :], z, ln2w_b, ln2b_b, eps_t, c)
                for k in range(KD):
                    pt = ptr3.tile([P, P], F32, name="pt3", tag="pt3")
                    nc.tensor.transpose(pt, z[:, P * k : P * (k + 1)], ident_t)
                    nc.scalar.copy(ZT[:, k, P * i : P * (i + 1)], pt)
                nc.vector.tensor_add(X[:, i, :], X[:, i, :], z)

        # ================= router, top-2 gate, b2-init of ACC =================
        gatep = ctx0.enter_context(tc.tile_pool(name="gatep", bufs=1))
        GATE = gatep.tile([P, TT, c.E], F32, name="GATE")
        GATET = gatep.tile([c.E, c.T], F32R, name="GATET")
        accp = ctx0.enter_context(tc.tile_pool(name="accp", bufs=1))
        ACC = accp.tile([P, TT, c.D], F32, name="ACC")
        with ExitStack() as rctx:
            rwp = rctx.enter_context(tc.tile_pool(name="rwp", bufs=1))
            RW = rwp.tile([P, KD, c.E], F32, name="RW")
            nc.sync.dma_start(RW, io["router_w"].rearrange("(k p) e -> p k e", p=P))
            B2 = rwp.tile([c.E, c.D], F32R, name="B2")
            nc.sync.dma_start(B2, io["b2"])
            rsmall = rctx.enter_context(tc.tile_pool(name="rsmall", bufs=4))
            prr = rctx.enter_context(tc.tile_pool(name="prr", bufs=2, space="PSUM"))
            ptg = rctx.enter_context(tc.tile_pool(name="ptg", bufs=2, space="PSUM"))
            pb2 = rctx.enter_context(tc.tile_pool(name="pb2", bufs=4, space="PSUM"))
            for i in range(TT):
                ps = prr.tile([P, c.E], F32, name="ps_r", tag="ps_r")
                for k in range(KD):
                    nc.tensor.matmul(
                        ps,
                        lhsT=ZT[:, k, P * i : P * (i + 1)].bitcast(F32),
                        rhs=RW[:, k, :],
                        start=(k == 0), stop=(k == KD - 1),
                    )
                mx = rsmall.tile([P, 1], F32, name="mx", tag="mx")
                nc.vector.reduce_max(mx, ps, axis=mybir.AxisListType.X)
                negmx = rsmall.tile([P, 1], F32, name="negmx", tag="negmx")
                nc.vector.tensor_scalar_mul(negmx, mx, -1.0)
                probs = rsmall.tile([P, c.E], F32, name="probs", tag="probs")
                sums = rsmall.tile([P, 1], F32, name="sums", tag="sums")
                nc.scalar.activation(
                    probs, ps, AF.Exp, bias=negmx, accum_out=sums
                )
                rcp = rsmall.tile([P, 1], F32, name="rcp", tag="rcp")
                nc.vector.reciprocal(rcp, sums)
                nc.vector.tensor_scalar_mul(probs, probs, rcp)
                m8 = rsmall.tile([P, 8], F32, name="m8", tag="m8")
                nc.vector.max(m8, probs)
                nc.vector.tensor_scalar(
                    out=GATE[:, i, :], in0=probs, scalar1=m8[:, 1:2], scalar2=None,
                    op0=OP.is_ge,
                )
                nc.vector.tensor_mul(GATE[:, i, :], GATE[:, i, :], probs)
                pt = ptg.tile([c.E, P], F32, name="ptg", tag="ptg")
                nc.tensor.transpose(pt, GATE[:, i, :], ident_t)
                nc.scalar.copy(GATET[:, P * i : P * (i + 1)], pt)
            # ACC = gate @ b2  (exact b2 contribution: sum_e gate[t,e] * b2[e,:])
            for i in range(TT):
                for off, sz in dch:
                    ps = pb2.tile([P, 512], F32, name="ps_b2", tag="ps_b2")
                    nc.tensor.matmul(
                        ps[:, :sz],
                        lhsT=GATET[:, P * i : P * (i + 1)],
                        rhs=B2[:, off : off + sz],
                        start=True, stop=True,
                    )
                    nc.any.tensor_copy(ACC[:, i, off : off + sz], ps[:, :sz])

        # ================= MoE experts =================
        with ExitStack() as mctx:
            w1p = mctx.enter_context(tc.tile_pool(name="w1p", bufs=1))
            w2p = mctx.enter_context(tc.tile_pool(name="w2p", bufs=1))
            gtp = mctx.enter_context(tc.tile_pool(name="gtp", bufs=4))
            ph1p = mctx.enter_context(tc.tile_pool(name="ph1p", bufs=2, space="PSUM"))
            pacc = mctx.enter_context(tc.tile_pool(name="pacc", bufs=4, space="PSUM"))
            DFFSTR = c.D * c.DFF  # elements per expert in w1

            for e in range(c.E):
                for dhalf in range(JSPLIT):
                    w1t = []
                    w2t = []
                    for j in range(JH):
                        jj = dhalf * JH + j
                        w1_ = w1p.tile([P, KD, P], F32R, name="w1t", tag="w1t", bufs=JH + 1)
                        # [p, k, cij] <- w1[e, 128k + p, 128jj + cij]
                        src = bass.AP(
                            tensor=io["w1"].tensor,
                            offset=e * DFFSTR + P * jj,
                            ap=[[c.DFF, P], [P * c.DFF, KD], [1, P]],
                        )
                        nc.sync.dma_start(w1_, src)
                        w1t.append(w1_)
                        w2_ = w2p.tile([P, c.D], F32R, name="w2t", tag="w2t", bufs=JH + 1)
                        nc.sync.dma_start(w2_, io["w2"][e, P * jj : P * (jj + 1), :])
                        w2t.append(w2_)
                    for half in range(NHALF):
                        # h1: all JH gelu tiles of this token-group materialized
                        gts = []
                        for j in range(JH):
                            jj = dhalf * JH + j
                            ph = ph1p.tile([P, QW], F32, name="ph1", tag="ph1")
                            for k in range(KD):
                                nc.tensor.matmul(
                                    ph,
                                    lhsT=w1t[j][:, k, :],
                                    rhs=ZT[:, k, QW * half : QW * (half + 1)],
                                    start=(k == 0), stop=(k == KD - 1),
                                )
                            g = gtp.tile([P, QW], F32R, name="g", tag="g", bufs=JH + 1)
                            nc.scalar.activation(
                                g, ph, gelu_af, bias=b1_sb[:, e, jj : jj + 1]
                            )
                            gts.append(g)
                        # h2: one PSUM accumulation group per (token-tile, chunk)
                        for i in range(TSUB):
                            ti = half * TSUB + i
                            for ci, (off, sz) in enumerate(dch):
                                ps = pacc.tile([P, 512], F32, name="pacc", tag="pacc")
                                for j in range(JH):
                                    nc.tensor.matmul(
                                        ps[:, :sz],
                                        lhsT=gts[j][:, P * i : P * (i + 1)],
                                        rhs=w2t[j][:, off : off + sz],
                                        start=(j == 0), stop=(j == JH - 1),
                                    )
                                nc.vector.scalar_tensor_tensor(
                                    out=ACC[:, ti, off : off + sz],
                                    in0=ps[:, :sz],
                                    scalar=GATE[:, ti, e : e + 1],
                                    in1=ACC[:, ti, off : off + sz],
                                    op0=OP.mult, op1=OP.add,
                                )

        # ================= out = r + yff =================
        with ExitStack() as octx:
            op = octx.enter_context(tc.tile_pool(name="outp", bufs=3))
            for i in range(TT):
                ot = op.tile([P, c.D], F32, name="ot", tag="ot")
                nc.vector.tensor_add(ot, X[:, i, :], ACC[:, i, :])
                nc.sync.dma_start(io["out"][P * i : P * (i + 1), :], ot)


def build(c: Cfg | None = None) -> bass.Bass:
    from concourse import bacc

    c = c or Cfg()
    nc = bacc.Bacc("TRN2", target_bir_lowering=False, debug=False)
    io = declare_io(nc, c)
    with tile.TileContext(nc) as tc:
        emit_block(tc, c, io)
    nc.compile()
    return nc


def make_consts(c: Cfg | None = None):
    c = c or Cfg()
    tri = np.triu(np.ones((P, P), np.float32))
    ident = np.eye(P, dtype=np.float32)
    return {"tri": tri, "ident": ident}


_BUILT: bass.Bass | None = None
_RUNNER = None

N_CORES = 8
_IN_NAMES = [
    "x", "ln1_w", "ln1_b", "qkv_w", "qkv_b", "proj_w", "proj_b",
    "ln2_w", "ln2_b", "router_w", "w1", "b1", "w2", "b2",
]


def get_runner(c: Cfg | None = None, n_cores: int = N_CORES):
    """Build (once) and return (fn, in_names, out_names, out_shapes).

    fn takes per-core-concatenated arrays (inputs then zero output buffers),
    runs the NEFF on n_cores devices via shard_map, returns output arrays.
    """
    global _BUILT, _RUNNER
    if _RUNNER is not None:
        return _RUNNER
    import jax
    from jax.experimental.shard_map import shard_map
    from jax.sharding import Mesh, PartitionSpec
    from concourse import bass2jax
    from concourse import mybir as _mb

    c = c or Cfg()
    if _BUILT is None:
        _BUILT = build(c)
    nc = _BUILT
    bass2jax.install_neuronx_cc_hook()
    assert nc.dbg_addr is None
    partition_name = nc.partition_id_tensor.name if nc.partition_id_tensor else None
    in_names, out_names, out_avals = [], [], []
    for alloc in nc.m.functions[0].allocations:
        if not isinstance(alloc, _mb.MemoryLocationSet):
            continue
        name = alloc.memorylocations[0].name
        if alloc.kind == "ExternalInput":
            if name != partition_name:
                in_names.append(name)
        elif alloc.kind == "ExternalOutput":
            out_avals.append(
                jax.core.ShapedArray(tuple(alloc.tensor_shape), _mb.dt.np(alloc.dtype))
            )
            out_names.append(name)
    n_params = len(in_names)
    all_in = tuple(in_names) + tuple(out_names)
    if partition_name is not None:
        all_in = all_in + (partition_name,)

    def _body(*args):
        ops = list(args)
        if partition_name is not None:
            ops.append(bass2jax.partition_id_tensor())
        outs = bass2jax._bass_exec_p.bind(
            *ops,
            out_avals=tuple(out_avals),
            in_names=all_in,
            out_names=tuple(out_names),
            lowering_input_output_aliases=(),
            sim_require_finite=True,
            sim_require_nnan=True,
            nc=nc,
        )
        return tuple(outs)

    _x_pos = in_names.index("x")
    _out_pos = out_names.index("out")

    def _chained_body(n_iter):
        def body(*args):
            args = list(args)
            for _ in range(n_iter):
                outs = _body(*args)
                args[_x_pos] = outs[_out_pos]
            return tuple(outs)
        return body

    devices = jax.devices()[:n_cores]
    mesh = Mesh(np.asarray(devices), ("core",))
    nio = n_params + len(out_names)

    def _make_fn(n_iter):
        return jax.jit(
            shard_map(
                _chained_body(n_iter),
                mesh=mesh,
                in_specs=(PartitionSpec("core"),) * nio,
                out_specs=(PartitionSpec("core"),) * len(out_names),
                check_rep=False,
            ),
            keep_unused=True,
        )

    fn = _make_fn(1)
    out_shapes = [tuple(a.shape) for a in out_avals]
    out_dtypes = [a.dtype for a in out_avals]
    _RUNNER = (fn, in_names, out_names, out_shapes, out_dtypes, _make_fn)
    return _RUNNER


def _concat_inputs(arrs, consts, c: Cfg, in_names, out_shapes, out_dtypes, n_cores=N_CORES):
    """Per-core replicated/sharded inputs, concatenated on axis 0 for shard_map."""
    x = arrs["x"]
    per_name = {}
    for nm in in_names:
        if nm == "x":
            per_name[nm] = np.ascontiguousarray(x.reshape(n_cores * c.T, c.D))
        else:
            src = consts[nm] if nm in consts else arrs[nm]
            per_name[nm] = np.concatenate([src] * n_cores, axis=0)
    ins = [per_name[nm] for nm in in_names]
    zouts = [
        np.zeros((n_cores * s[0], *s[1:]), dt) for s, dt in zip(out_shapes, out_dtypes)
    ]
    return ins, zouts


def kernel(**inputs) -> np.ndarray:
    c = Cfg()
    arrs = {
        k: np.ascontiguousarray(np.asarray(v, dtype=np.float32))
        for k, v in inputs.items()
    }
    x = arrs["x"]  # [B, T, D]
    B = x.shape[0]
    assert B == N_CORES and x.shape[1] == c.T and x.shape[2] == c.D

    fn, in_names, out_names, out_shapes, out_dtypes, _mk = get_runner(c)
    consts = make_consts(c)
    ins, zouts = _concat_inputs(arrs, consts, c, in_names, out_shapes, out_dtypes)
    outs = fn(*ins, *zouts)
    out = np.asarray(outs[out_names.index("out")]).reshape(N_CORES, c.T, c.D)
    return out.astype(np.float32)


def _warmup():
    """Compile the NEFF + load executables at import so kernel() calls are fast."""
    try:
        c = Cfg()
        fn, in_names, out_names, out_shapes, out_dtypes, _mk = get_runner(c)
        rng = np.random.default_rng(0)
        dummy = {
            "x": np.zeros((N_CORES, c.T, c.D), np.float32),
            "ln1_w": np.ones(c.D, np.float32), "ln1_b": np.zeros(c.D, np.float32),
            "qkv_w": np.zeros((c.D, 3 * c.D), np.float32),
            "qkv_b": np.zeros(3 * c.D, np.float32),
            "proj_w": np.zeros((c.D, c.D), np.float32),
            "proj_b": np.zeros(c.D, np.float32),
            "ln2_w": np.ones(c.D, np.float32), "ln2_b": np.zeros(c.D, np.float32),
            "router_w": np.zeros((c.D, c.E), np.float32),
            "w1": np.zeros((c.E, c.D, c.DFF), np.float32),
            "b1": np.zeros((c.E, c.DFF), np.float32),
            "w2": np.zeros((c.E, c.DFF, c.D), np.float32),
            "b2": np.zeros((c.E, c.D), np.float32),
        }
        consts = make_consts(c)
        ins, zouts = _concat_inputs(dummy, consts, c, in_names, out_shapes, out_dtypes)
        import jax
        jax.block_until_ready(fn(*ins, *zouts))
    except Exception:
        import traceback
        traceback.print_exc()


import os as _os

if not _os.environ.get("KERNEL_NO_WARMUP"):
    _warmup()


# revision 10
# speedup vs baseline: 23.8915x; 1.0043x over previous
"""Trainium2 Bass kernel: transformer block (causal MHA + dense top-2-gated MoE FFN).

Problem: nn_Block_24541443129820  (B=8, T=1024, D=768, H=12, DH=64, E=16, DFF=3072)

Sharding: data-parallel over batch. B == n_cores == 8, so each NeuronCore runs
the complete block (LN1 -> QKV -> causal attention -> proj+residual -> LN2 ->
router/top-2 gate -> all 16 experts, dense -> gated combine + double residual)
on one [1024, 768] batch slice. Weights are replicated to every core; the host
splits x on batch and stacks the per-core outputs. This is perfectly load
balanced and needs no collectives.

Per-core layout strategy:
 - activations live in "N-layout" [token-partition, feature-free] for LN/softmax
   (free-dim reductions) and in "T-layout" [feature-partition, token-free] when
   they feed matmuls as the stationary operand; PE transposes bridge the two.
 - all large matmuls run as float32r (1 PE cycle/row for free dim >= 256, i.e.
   bf16 speed with ~tf32 precision); attention probabilities/V and the router
   stay full fp32 so top-2 expert selection matches the fp32 reference.
 - expert MLPs: h1 = w1[e].T-slices x zT -> PSUM -> Gelu(+b1) on ACT -> gT in
   T-layout feeds h2 which accumulates K=DFF in PSUM; gate applied on eviction
   via one fused DVE scalar_tensor_tensor per chunk: acc += gate[t,e] * psum.
   The b2 term is folded in exactly via a tiny gateT @ b2 matmul that
   initializes the accumulator.
"""
import math
import sys

for _p in ("/opt/trn_rl_repo", "/root/.axon_site/_ro/trn_rl_repo"):
    if _p not in sys.path:
        sys.path.append(_p)

from contextlib import ExitStack
from dataclasses import dataclass

import numpy as np

import concourse.bass as bass
import concourse.tile as tile
from concourse import mybir

AF = mybir.ActivationFunctionType
OP = mybir.AluOpType
F32 = mybir.dt.float32
F32R = mybir.dt.float32r
BF16 = mybir.dt.bfloat16
P = 128


@dataclass(frozen=True)
class Cfg:
    T: int = 1024
    D: int = 768
    H: int = 12
    DH: int = 64
    E: int = 16
    DFF: int = 3072
    eps: float = 1e-5
    exact_gelu: bool = True  # False -> Tanh in place of Gelu (CoreSim lacks Gelu)


def _chunks(n, step=512):
    out = []
    off = 0
    while off < n:
        sz = min(step, n - off)
        out.append((off, sz))
        off += sz
    return out


def _bcast_ap(src_1d, parts):
    """DRAM [N] -> AP that a DMA reads as [parts, N] (partition-replicated)."""
    return bass.AP(
        tensor=src_1d.tensor,
        offset=src_1d.offset,
        ap=[[0, parts]] + [list(d) for d in src_1d.ap],
    )


def declare_io(nc: bass.Bass, c: Cfg):
    D3 = 3 * c.D
    io = {
        "x": nc.dram_tensor("x", [c.T, c.D], F32, kind="ExternalInput").ap(),
        "ln1_w": nc.dram_tensor("ln1_w", [c.D], F32, kind="ExternalInput").ap(),
        "ln1_b": nc.dram_tensor("ln1_b", [c.D], F32, kind="ExternalInput").ap(),
        "qkv_w": nc.dram_tensor("qkv_w", [c.D, D3], F32R, kind="ExternalInput").ap(),
        "qkv_b": nc.dram_tensor("qkv_b", [D3], F32, kind="ExternalInput").ap(),
        "proj_w": nc.dram_tensor("proj_w", [c.D, c.D], F32R, kind="ExternalInput").ap(),
        "proj_b": nc.dram_tensor("proj_b", [c.D], F32, kind="ExternalInput").ap(),
        "ln2_w": nc.dram_tensor("ln2_w", [c.D], F32, kind="ExternalInput").ap(),
        "ln2_b": nc.dram_tensor("ln2_b", [c.D], F32, kind="ExternalInput").ap(),
        "router_w": nc.dram_tensor("router_w", [c.D, c.E], F32, kind="ExternalInput").ap(),
        "w1": nc.dram_tensor("w1", [c.E, c.D, c.DFF], F32R, kind="ExternalInput").ap(),
        "b1": nc.dram_tensor("b1", [c.E, c.DFF], F32, kind="ExternalInput").ap(),
        "w2": nc.dram_tensor("w2", [c.E, c.DFF, c.D], F32R, kind="ExternalInput").ap(),
        "b2": nc.dram_tensor("b2", [c.E, c.D], F32R, kind="ExternalInput").ap(),
        "tri": nc.dram_tensor("tri", [P, P], F32, kind="ExternalInput").ap(),
        "ident": nc.dram_tensor("ident", [P, P], F32, kind="ExternalInput").ap(),
        "out": nc.dram_tensor("out", [c.T, c.D], F32, kind="ExternalOutput").ap(),
    }
    return io


def _emit_ln(nc, stat, src, dst, w_b, b_b, eps_t, c):
    """dst = LN(src) * w + b, rowwise over the free dim (size D)."""
    SG = math.gcd(512, c.D)
    NSG = c.D // SG
    st = stat.tile([P, NSG, 6], F32, name="bnst", tag="bnst")
    for s in range(NSG):
        nc.vector.bn_stats(st[:, s, :], src[:, SG * s : SG * (s + 1)])
    mv = stat.tile([P, 2], F32, name="bnmv", tag="bnmv")
    nc.vector.bn_aggr(mv, st)
    rstd = stat.tile([P, 1], F32, name="rstd", tag="rstd")
    nc.scalar.activation(rstd, mv[:, 1:2], AF.Sqrt, bias=eps_t)
    nc.vector.reciprocal(rstd, rstd)
    nc.vector.tensor_scalar(
        out=dst, in0=src, scalar1=mv[:, 0:1], scalar2=rstd,
        op0=OP.subtract, op1=OP.mult,
    )
    nc.vector.tensor_mul(dst, dst, w_b)
    nc.vector.tensor_add(dst, dst, b_b)


def emit_block(tc: tile.TileContext, c: Cfg, io):
    nc = tc.nc
    TT = c.T // P           # token tiles
    KD = c.D // P           # model-dim k-tiles
    JD = c.DFF // P         # dff tiles
    JSPLIT = 2 if JD % 2 == 0 and JD > 1 else 1
    JH = JD // JSPLIT       # dff tiles per dff-half
    QW = min(512, c.T)      # MoE token-group width
    NHALF = c.T // QW
    TSUB = QW // P
    NQ = min(512, c.T)      # attention q-chunk width
    NCH = c.T // NQ
    HPT = P // c.DH         # heads per qT/kT partition tile
    QKT = (c.H * c.DH) // P  # qT (or kT) partition tiles
    DQK = 2 * c.H * c.DH
    dch = _chunks(c.D, 512)
    gelu_af = AF.Gelu if c.exact_gelu else AF.Tanh
    assert c.T % P == 0 and c.D % P == 0 and c.DFF % P == 0
    assert (c.H * c.DH) % P == 0 and c.DH <= P and P % c.DH == 0
    assert all(sz % c.DH == 0 for _, sz in dch)
    assert c.E >= 8  # vector.max needs >= 8 candidates

    with ExitStack() as ctx0:
        const = ctx0.enter_context(tc.tile_pool(name="const", bufs=1))
        ident_t = const.tile([P, P], F32, name="ident_t")
        nc.sync.dma_start(ident_t, io["ident"])
        tri_t = const.tile([P, P], F32, name="tri_t")
        nc.sync.dma_start(tri_t, io["tri"])
        eps_t = const.tile([P, 1], F32, name="eps_t")
        nc.vector.memset(eps_t, c.eps)

        # b1 as per-partition columns: [P, e, j] = b1[e, 128j + p]
        b1_sb = const.tile([P, c.E, JD], F32, name="b1_sb")
        nc.sync.dma_start(b1_sb, io["b1"].rearrange("e (j p) -> p e j", p=P))

        persistX = ctx0.enter_context(tc.tile_pool(name="persistX", bufs=1))
        X = persistX.tile([P, TT, c.D], F32, name="X")
        for i in range(TT):
            nc.sync.dma_start(X[:, i, :], io["x"][P * i : P * (i + 1), :])

        # ================= attention =================
        with ExitStack() as actx:
            aouter = actx.enter_context(tc.tile_pool(name="attn_outer", bufs=1))
            QT = aouter.tile([P, QKT, c.T], F32R, name="QT")
            KTt = aouter.tile([P, QKT, c.T], F32R, name="KTt")
            VEXT = aouter.tile([P, TT, c.H, c.DH + 1], F32, name="VEXT")
            Y = aouter.tile([P, TT, c.D], F32, name="Y")

            # ---- LN1 + transpose h -> hT + QKV matmuls ----
            with ExitStack() as qctx:
                hpool = qctx.enter_context(tc.tile_pool(name="hpool", bufs=3))
                htp = qctx.enter_context(tc.tile_pool(name="htp", bufs=1))
                HT = htp.tile([P, KD, c.T], F32R, name="HT")
                wpool = qctx.enter_context(tc.tile_pool(name="qkvwp", bufs=1))
                stat = qctx.enter_context(tc.tile_pool(name="stat1", bufs=4))
                cst1 = qctx.enter_context(tc.tile_pool(name="cst1", bufs=1))
                ln1w_b = cst1.tile([P, c.D], F32, name="ln1w_b")
                nc.gpsimd.dma_start(ln1w_b, _bcast_ap(io["ln1_w"], P))
                ln1b_b = cst1.tile([P, c.D], F32, name="ln1b_b")
                nc.gpsimd.dma_start(ln1b_b, _bcast_ap(io["ln1_b"], P))
                vbias_b = cst1.tile([P, c.D], F32, name="vbias_b")
                nc.gpsimd.dma_start(vbias_b, _bcast_ap(io["qkv_b"][DQK : DQK + c.D], P))
                # qkv_b for q,k as per-partition columns: col j = qkv_b[128j:128(j+1)]
                qkvbT = cst1.tile([P, DQK // P], F32, name="qkvbT")
                nc.sync.dma_start(qkvbT, io["qkv_b"][0:DQK].rearrange("(j p) -> p j", p=P))
                ptr = qctx.enter_context(tc.tile_pool(name="ptr1", bufs=4, space="PSUM"))
                pmm = qctx.enter_context(tc.tile_pool(name="pmm1", bufs=4, space="PSUM"))

                for i in range(TT):
                    h = hpool.tile([P, c.D], F32, name="h", tag="h")
                    _emit_ln(nc, stat, X[:, i, :], h, ln1w_b, ln1b_b, eps_t, c)
                    for k in range(KD):
                        pt = ptr.tile([P, P], F32, name="pt1", tag="pt1")
                        nc.tensor.transpose(pt, h[:, P * k : P * (k + 1)], ident_t)
                        nc.scalar.copy(HT[:, k, P * i : P * (i + 1)], pt)

                # qT / kT: out[dout_tile, tq] = qkv_w[:, tile].T @ hT
                for j in range(DQK // P):
                    wcol = []
                    for k in range(KD):
                        wt = wpool.tile([P, P], F32R, name="wqk", tag="wqk", bufs=KD + 2)
                        nc.sync.dma_start(
                            wt, io["qkv_w"][P * k : P * (k + 1), P * j : P * (j + 1)]
                        )
                        wcol.append(wt)
                    dst = QT if j < QKT else KTt
                    jj = j % QKT
                    for cc in range(NCH):
                        ps = pmm.tile([P, NQ], F32, name="ps_qk", tag="ps_qk")
                        for k in range(KD):
                            nc.tensor.matmul(
                                ps,
                                lhsT=wcol[k],
                                rhs=HT[:, k, NQ * cc : NQ * (cc + 1)],
                                start=(k == 0), stop=(k == KD - 1),
                            )
                        nc.scalar.activation(
                            dst[:, jj, NQ * cc : NQ * (cc + 1)], ps, AF.Identity,
                            bias=qkvbT[:, j : j + 1],
                        )

                # v (+bias) in N-layout, scattered into VEXT[:, :, h, 0:DH]
                for off, sz in dch:
                    wv = []
                    for k in range(KD):
                        wt = wpool.tile([P, 512], F32R, name="wv", tag="wv", bufs=KD + 2)
                        nc.sync.dma_start(
                            wt[:, :sz],
                            io["qkv_w"][P * k : P * (k + 1), DQK + off : DQK + off + sz],
                        )
                        wv.append(wt)
                    nh = sz // c.DH
                    h0 = off // c.DH
                    for i in range(TT):
                        ps = pmm.tile([P, sz], F32, name="ps_v", tag="ps_qk")
                        for k in range(KD):
                            nc.tensor.matmul(
                                ps,
                                lhsT=HT[:, k, P * i : P * (i + 1)],
                                rhs=wv[k][:, :sz],
                                start=(k == 0), stop=(k == KD - 1),
                            )
                        nc.vector.tensor_add(
                            VEXT[:, i, h0 : h0 + nh, 0 : c.DH],
                            ps.rearrange("p (h d) -> p h d", d=c.DH),
                            vbias_b[:, off : off + sz].rearrange(
                                "p (h d) -> p h d", d=c.DH
                            ),
                        )
                nc.vector.memset(VEXT[:, :, :, c.DH : c.DH + 1], 1.0)

            # ---- heads: scoresT -> exp -> causal mask -> A@V (+sums) ----
            with ExitStack() as hctx:
                apool = hctx.enter_context(tc.tile_pool(name="apool", bufs=1))
                small = hctx.enter_context(tc.tile_pool(name="asmall", bufs=6))
                pscore = hctx.enter_context(
                    tc.tile_pool(name="pscore", bufs=2, space="PSUM")
                )
                pav = hctx.enter_context(tc.tile_pool(name="pav", bufs=4, space="PSUM"))
                inv_sqrt_dh = 1.0 / math.sqrt(c.DH)
                for hh in range(c.H):
                    at = apool.tile([P, TT, c.T], F32, name="at", tag="at")
                    pt_i = hh // HPT
                    po = (hh % HPT) * c.DH
                    for t in range(TT):
                        for cc in range(NCH):
                            if NQ * (cc + 1) <= P * t:
                                continue  # chunk fully in the causal-masked region
                            ps = pscore.tile([P, NQ], F32, name="ps_s", tag="ps_s")
                            nc.tensor.matmul(
                                ps,
                                lhsT=KTt[po : po + c.DH, pt_i, P * t : P * (t + 1)],
                                rhs=QT[po : po + c.DH, pt_i, NQ * cc : NQ * (cc + 1)],
                                start=True, stop=True,
                            )
                            nc.scalar.activation(
                                at[:, t, NQ * cc : NQ * (cc + 1)], ps, AF.Exp,
                                scale=inv_sqrt_dh,
                            )
                        # diagonal 128x128 block: zero out k > q
                        nc.vector.tensor_mul(
                            at[:, t, P * t : P * (t + 1)],
                            at[:, t, P * t : P * (t + 1)],
                            tri_t,
                        )
                    for i in range(TT):
                        pv = pav.tile([P, c.DH + 1], F32, name="pv", tag="pv")
                        for t in range(i + 1):
                            nc.tensor.matmul(
                                pv,
                                lhsT=at[:, t, P * i : P * (i + 1)],
                                rhs=VEXT[:, t, hh, :],
                                start=(t == 0), stop=(t == i),
                            )
                        rc = small.tile([P, 1], F32, name="rc", tag="rc")
                        nc.vector.reciprocal(rc, pv[:, c.DH : c.DH + 1])
                        nc.scalar.activation(
                            Y[:, i, c.DH * hh : c.DH * (hh + 1)], pv[:, 0 : c.DH],
                            AF.Copy, scale=rc,
                        )

            # ---- y -> ynT, proj, residual into X ----
            with ExitStack() as pctx:
                ynp = pctx.enter_context(tc.tile_pool(name="ynp", bufs=1))
                YNT = ynp.tile([P, KD, c.T], F32R, name="YNT")
                pwpool = pctx.enter_context(tc.tile_pool(name="pwpool", bufs=1))
                ptr2 = pctx.enter_context(tc.tile_pool(name="ptr2", bufs=4, space="PSUM"))
                cst2 = pctx.enter_context(tc.tile_pool(name="cst2", bufs=1))
                projb_b = cst2.tile([P, c.D], F32, name="projb_b")
                nc.gpsimd.dma_start(projb_b, _bcast_ap(io["proj_b"], P))
                pmm2 = pctx.enter_context(tc.tile_pool(name="pmm2", bufs=4, space="PSUM"))
                for i in range(TT):
                    for k in range(KD):
                        pt = ptr2.tile([P, P], F32, name="pt2", tag="pt2")
                        nc.tensor.transpose(pt, Y[:, i, P * k : P * (k + 1)], ident_t)
                        nc.scalar.copy(YNT[:, k, P * i : P * (i + 1)], pt)
                    nc.vector.tensor_add(X[:, i, :], X[:, i, :], projb_b)
                for off, sz in dch:
                    pw = []
                    for k in range(KD):
                        wt = pwpool.tile([P, 512], F32R, name="pw", tag="pw", bufs=KD + 2)
                        nc.sync.dma_start(
                            wt[:, :sz],
                            io["proj_w"][P * k : P * (k + 1), off : off + sz],
                        )
                        pw.append(wt)
                    for i in range(TT):
                        ps = pmm2.tile([P, sz], F32, name="ps_p", tag="ps_p")
                        for k in range(KD):
                            nc.tensor.matmul(
                                ps,
                                lhsT=YNT[:, k, P * i : P * (i + 1)],
                                rhs=pw[k][:, :sz],
                                start=(k == 0), stop=(k == KD - 1),
                            )
                        nc.vector.scalar_tensor_tensor(
                            out=X[:, i, off : off + sz], in0=ps, scalar=1.0,
                            in1=X[:, i, off : off + sz],
                            op0=OP.mult, op1=OP.add,
                        )

        # ================= LN2 -> zT; r = x1 + z into X =================
        ztp = ctx0.enter_context(tc.tile_pool(name="ztp", bufs=1))
        ZT = ztp.tile([P, KD, c.T], F32R, name="ZT")
        with ExitStack() as lctx:
            zpool = lctx.enter_context(tc.tile_pool(name="zpool", bufs=3))
            stat2 = lctx.enter_context(tc.tile_pool(name="stat2", bufs=4))
            ptr3 = lctx.enter_context(tc.tile_pool(name="ptr3", bufs=4, space="PSUM"))
            cst3 = lctx.enter_context(tc.tile_pool(name="cst3", bufs=1))
            ln2w_b = cst3.tile([P, c.D], F32, name="ln2w_b")
            nc.gpsimd.dma_start(ln2w_b, _bcast_ap(io["ln2_w"], P))
            ln2b_b = cst3.tile([P, c.D], F32, name="ln2b_b")
            nc.gpsimd.dma_start(ln2b_b, _bcast_ap(io["ln2_b"], P))
            for i in range(TT):
                z = zpool.tile([P, c.D], F32, name="z", tag="z")
                _emit_ln(nc, stat2, X[:, i, :], z, ln2w_b, ln2b_b, eps_t, c)
                for k in range(KD):
                    pt = ptr3.tile([P, P], F32, name="pt3", tag="pt3")
                    nc.tensor.transpose(pt, z[:, P * k : P * (k + 1)], ident_t)
                    nc.scalar.copy(ZT[:, k, P * i : P * (i + 1)], pt)
                nc.vector.tensor_add(X[:, i, :], X[:, i, :], z)

        # ================= router, top-2 gate, b2-init of ACC =================
        gatep = ctx0.enter_context(tc.tile_pool(name="gatep", bufs=1))
        GATE = gatep.tile([P, TT, c.E], F32, name="GATE")
        GATET = gatep.tile([c.E, c.T], F32R, name="GATET")
        accp = ctx0.enter_context(tc.tile_pool(name="accp", bufs=1))
        ACC = accp.tile([P, TT, c.D], F32, name="ACC")
        with ExitStack() as rctx:
            rwp = rctx.enter_context(tc.tile_pool(name="rwp", bufs=1))
            RW = rwp.tile([P, KD, c.E], F32, name="RW")
            nc.sync.dma_start(RW, io["router_w"].rearrange("(k p) e -> p k e", p=P))
            B2 = rwp.tile([c.E, c.D], F32R, name="B2")
            nc.sync.dma_start(B2, io["b2"])
            rsmall = rctx.enter_context(tc.tile_pool(name="rsmall", bufs=4))
            prr = rctx.enter_context(tc.tile_pool(name="prr", bufs=2, space="PSUM"))
            ptg = rctx.enter_context(tc.tile_pool(name="ptg", bufs=2, space="PSUM"))
            pb2 = rctx.enter_context(tc.tile_pool(name="pb2", bufs=4, space="PSUM"))
            for i in range(TT):
                ps = prr.tile([P, c.E], F32, name="ps_r", tag="ps_r")
                for k in range(KD):
                    nc.tensor.matmul(
                        ps,
                        lhsT=ZT[:, k, P * i : P * (i + 1)].bitcast(F32),
                        rhs=RW[:, k, :],
                        start=(k == 0), stop=(k == KD - 1),
                    )
                mx = rsmall.tile([P, 1], F32, name="mx", tag="mx")
                nc.vector.reduce_max(mx, ps, axis=mybir.AxisListType.X)
                negmx = rsmall.tile([P, 1], F32, name="negmx", tag="negmx")
                nc.vector.tensor_scalar_mul(negmx, mx, -1.0)
                probs = rsmall.tile([P, c.E], F32, name="probs", tag="probs")
                sums = rsmall.tile([P, 1], F32, name="sums", tag="sums")
                nc.scalar.activation(
                    probs, ps, AF.Exp, bias=negmx, accum_out=sums
                )
                rcp = rsmall.tile([P, 1], F32, name="rcp", tag="rcp")
                nc.vector.reciprocal(rcp, sums)
                nc.vector.tensor_scalar_mul(probs, probs, rcp)
                m8 = rsmall.tile([P, 8], F32, name="m8", tag="m8")
                nc.vector.max(m8, probs)
                nc.vector.tensor_scalar(
                    out=GATE[:, i, :], in0=probs, scalar1=m8[:, 1:2], scalar2=None,
                    op0=OP.is_ge,
                )
                nc.vector.tensor_mul(GATE[:, i, :], GATE[:, i, :], probs)
                pt = ptg.tile([c.E, P], F32, name="ptg", tag="ptg")
                nc.tensor.transpose(pt, GATE[:, i, :], ident_t)
                nc.scalar.copy(GATET[:, P * i : P * (i + 1)], pt)
            # ACC = gate @ b2  (exact b2 contribution: sum_e gate[t,e] * b2[e,:])
            for i in range(TT):
                for off, sz in dch:
                    ps = pb2.tile([P, 512], F32, name="ps_b2", tag="ps_b2")
                    nc.tensor.matmul(
                        ps[:, :sz],
                        lhsT=GATET[:, P * i : P * (i + 1)],
                        rhs=B2[:, off : off + sz],
                        start=True, stop=True,
                    )
                    nc.any.tensor_copy(ACC[:, i, off : off + sz], ps[:, :sz])

        # ================= MoE experts =================
        with ExitStack() as mctx:
            w1p = mctx.enter_context(tc.tile_pool(name="w1p", bufs=1))
            w2p = mctx.enter_context(tc.tile_pool(name="w2p", bufs=1))
            gtp = mctx.enter_context(tc.tile_pool(name="gtp", bufs=4))
            ph1p = mctx.enter_context(tc.tile_pool(name="ph1p", bufs=2, space="PSUM"))
            pacc = mctx.enter_context(tc.tile_pool(name="pacc", bufs=4, space="PSUM"))
            DFFSTR = c.D * c.DFF  # elements per expert in w1

            for e in range(c.E):
                for dhalf in range(JSPLIT):
                    w1t = []
                    w2t = []
                    for j in range(JH):
                        jj = dhalf * JH + j
                        w1_ = w1p.tile([P, KD, P], F32R, name="w1t", tag="w1t", bufs=JH + 1)
                        # [p, k, cij] <- w1[e, 128k + p, 128jj + cij]
                        src = bass.AP(
                            tensor=io["w1"].tensor,
                            offset=e * DFFSTR + P * jj,
                            ap=[[c.DFF, P], [P * c.DFF, KD], [1, P]],
                        )
                        nc.sync.dma_start(w1_, src)
                        w1t.append(w1_)
                        w2_ = w2p.tile([P, c.D], F32R, name="w2t", tag="w2t", bufs=JH + 1)
                        nc.sync.dma_start(w2_, io["w2"][e, P * jj : P * (jj + 1), :])
                        w2t.append(w2_)
                    for half in range(NHALF):
                        # h1: all JH gelu tiles of this token-group materialized
                        gts = []
                        for j in range(JH):
                            jj = dhalf * JH + j
                            ph = ph1p.tile([P, QW], F32, name="ph1", tag="ph1")
                            for k in range(KD):
                                nc.tensor.matmul(
                                    ph,
                                    lhsT=w1t[j][:, k, :],
                                    rhs=ZT[:, k, QW * half : QW * (half + 1)],
                                    start=(k == 0), stop=(k == KD - 1),
                                )
                            g = gtp.tile([P, QW], F32R, name="g", tag="g", bufs=JH + 1)
                            nc.scalar.activation(
                                g, ph, gelu_af, bias=b1_sb[:, e, jj : jj + 1]
                            )
                            gts.append(g)
                        # h2: one PSUM accumulation group per (token-tile, chunk)
                        for i in range(TSUB):
                            ti = half * TSUB + i
                            for ci, (off, sz) in enumerate(dch):
                                ps = pacc.tile([P, 512], F32, name="pacc", tag="pacc")
                                for j in range(JH):
                                    nc.tensor.matmul(
                                        ps[:, :sz],
                                        lhsT=gts[j][:, P * i : P * (i + 1)],
                                        rhs=w2t[j][:, off : off + sz],
                                        start=(j == 0), stop=(j == JH - 1),
                                    )
                                nc.vector.scalar_tensor_tensor(
                                    out=ACC[:, ti, off : off + sz],
                                    in0=ps[:, :sz],
                                    scalar=GATE[:, ti, e : e + 1],
                                    in1=ACC[:, ti, off : off + sz],
                                    op0=OP.mult, op1=OP.add,
                                )

        # ================= out = r + yff =================
        with ExitStack() as octx:
            op = octx.enter_context(tc.tile_pool(name="outp", bufs=3))
            for i in range(TT):
                ot = op.tile([P, c.D], F32, name="ot", tag="ot")
                nc.vector.tensor_add(ot, X[:, i, :], ACC[:, i, :])
                nc.sync.dma_start(io["out"][P * i : P * (i + 1), :], ot)


def build(c: Cfg | None = None) -> bass.Bass:
    from concourse import bacc

    c = c or Cfg()
    nc = bacc.Bacc("TRN2", target_bir_lowering=False, debug=False)
    io = declare_io(nc, c)
    with tile.TileContext(nc) as tc:
        emit_block(tc, c, io)
    nc.compile()
    return nc


def make_consts(c: Cfg | None = None):
    c = c or Cfg()
    tri = np.triu(np.ones((P, P), np.float32))
    ident = np.eye(P, dtype=np.float32)
    return {"tri": tri, "ident": ident}


_BUILT: bass.Bass | None = None
_RUNNER = None

N_CORES = 8
_IN_NAMES = [
    "x", "ln1_w", "ln1_b", "qkv_w", "qkv_b", "proj_w", "proj_b",
    "ln2_w", "ln2_b", "router_w", "w1", "b1", "w2", "b2",
]


def get_runner(c: Cfg | None = None, n_cores: int = N_CORES):
    """Build (once) and return (fn, in_names, out_names, out_shapes).

    fn takes per-core-concatenated arrays (inputs then zero output buffers),
    runs the NEFF on n_cores devices via shard_map, returns output arrays.
    """
    global _BUILT, _RUNNER
    if _RUNNER is not None:
        return _RUNNER
    import jax
    from jax.experimental.shard_map import shard_map
    from jax.sharding import Mesh, PartitionSpec
    from concourse import bass2jax
    from concourse import mybir as _mb

    c = c or Cfg()
    if _BUILT is None:
        _BUILT = build(c)
    nc = _BUILT
    bass2jax.install_neuronx_cc_hook()
    assert nc.dbg_addr is None
    partition_name = nc.partition_id_tensor.name if nc.partition_id_tensor else None
    in_names, out_names, out_avals = [], [], []
    for alloc in nc.m.functions[0].allocations:
        if not isinstance(alloc, _mb.MemoryLocationSet):
            continue
        name = alloc.memorylocations[0].name
        if alloc.kind == "ExternalInput":
            if name != partition_name:
                in_names.append(name)
        elif alloc.kind == "ExternalOutput":
            out_avals.append(
                jax.core.ShapedArray(tuple(alloc.tensor_shape), _mb.dt.np(alloc.dtype))
            )
            out_names.append(name)
    n_params = len(in_names)
    all_in = tuple(in_names) + tuple(out_names)
    if partition_name is not None:
        all_in = all_in + (partition_name,)

    def _body(*args):
        ops = list(args)
        if partition_name is not None:
            ops.append(bass2jax.partition_id_tensor())
        outs = bass2jax._bass_exec_p.bind(
            *ops,
            out_avals=tuple(out_avals),
            in_names=all_in,
            out_names=tuple(out_names),
            lowering_input_output_aliases=(),
            sim_require_finite=True,
            sim_require_nnan=True,
            nc=nc,
        )
        return tuple(outs)

    _x_pos = in_names.index("x")
    _out_pos = out_names.index("out")

    def _chained_body(n_iter):
        def body(*args):
            args = list(args)
            for _ in range(n_iter):
                outs = _body(*args)
                args[_x_pos] = outs[_out_pos]
            return tuple(outs)
        return body

    devices = jax.devices()[:n_cores]
    mesh = Mesh(np.asarray(devices), ("core",))
    nio = n_params + len(out_names)

    def _make_fn(n_iter):
        return jax.jit(
            shard_map(
                _chained_body(n_iter),
                mesh=mesh,
                in_specs=(PartitionSpec("core"),) * nio,
                out_specs=(PartitionSpec("core"),) * len(out_names),
                check_rep=False,
            ),
            keep_unused=True,
        )

    fn = _make_fn(1)
    out_shapes = [tuple(a.shape) for a in out_avals]
    out_dtypes = [a.dtype for a in out_avals]
    _RUNNER = (fn, in_names, out_names, out_shapes, out_dtypes, _make_fn)
    return _RUNNER


def _concat_inputs(arrs, consts, c: Cfg, in_names, out_shapes, out_dtypes, n_cores=N_CORES):
    """Per-core replicated/sharded inputs, concatenated on axis 0 for shard_map."""
    x = arrs["x"]
    per_name = {}
    for nm in in_names:
        if nm == "x":
            per_name[nm] = np.ascontiguousarray(x.reshape(n_cores * c.T, c.D))
        else:
            src = consts[nm] if nm in consts else arrs[nm]
            per_name[nm] = np.concatenate([src] * n_cores, axis=0)
    ins = [per_name[nm] for nm in in_names]
    zouts = [
        np.zeros((n_cores * s[0], *s[1:]), dt) for s, dt in zip(out_shapes, out_dtypes)
    ]
    return ins, zouts


def kernel(**inputs) -> np.ndarray:
    c = Cfg()
    arrs = {
        k: np.ascontiguousarray(np.asarray(v, dtype=np.float32))
        for k, v in inputs.items()
    }
    x = arrs["x"]  # [B, T, D]
    B = x.shape[0]
    assert B == N_CORES and x.shape[1] == c.T and x.shape[2] == c.D

    fn, in_names, out_names, out_shapes, out_dtypes, _mk = get_runner(c)
    consts = make_consts(c)
    ins, zouts = _concat_inputs(arrs, consts, c, in_names, out_shapes, out_dtypes)
    outs = fn(*ins, *zouts)
    out = np.asarray(outs[out_names.index("out")]).reshape(N_CORES, c.T, c.D)
    return out.astype(np.float32)


def _warmup():
    """Compile the NEFF + load executables at import so kernel() calls are fast."""
    try:
        c = Cfg()
        fn, in_names, out_names, out_shapes, out_dtypes, _mk = get_runner(c)
        rng = np.random.default_rng(0)
        dummy = {
            "x": np.zeros((N_CORES, c.T, c.D), np.float32),
            "ln1_w": np.ones(c.D, np.float32), "ln1_b": np.zeros(c.D, np.float32),
            "qkv_w": np.zeros((c.D, 3 * c.D), np.float32),
            "qkv_b": np.zeros(3 * c.D, np.float32),
            "proj_w": np.zeros((c.D, c.D), np.float32),
            "proj_b": np.zeros(c.D, np.float32),
            "ln2_w": np.ones(c.D, np.float32), "ln2_b": np.zeros(c.D, np.float32),
            "router_w": np.zeros((c.D, c.E), np.float32),
            "w1": np.zeros((c.E, c.D, c.DFF), np.float32),
            "b1": np.zeros((c.E, c.DFF), np.float32),
            "w2": np.zeros((c.E, c.DFF, c.D), np.float32),
            "b2": np.zeros((c.E, c.D), np.float32),
        }
        consts = make_consts(c)
        ins, zouts = _concat_inputs(dummy, consts, c, in_names, out_shapes, out_dtypes)
        import jax
        jax.block_until_ready(fn(*ins, *zouts))
    except Exception:
        import traceback
        traceback.print_exc()


import os as _os

if not _os.environ.get("KERNEL_NO_WARMUP"):
    _warmup()
